# revision 10
# baseline (speedup 1.0000x reference)
"""Multi-head attention Trainium2 kernel (8 NeuronCores, SPMD).

Sharding: core c handles batch b = c//4 and heads [4*(c%4), 4*(c%4)+4).
Each core computes Q/K/V projections for its 4 heads, causal+biased
softmax attention, and a partial out-projection (its heads' columns of
wo). Host sums the 4 partials per batch and adds bo.

Device layout choices:
  - Scores are computed TRANSPOSED: S^T[j, i] (j = key pos on partitions,
    i = query pos on free dim).  attn_bias is transposed on the host so
    its tiles load contiguously.  The PV matmul then needs no on-chip
    transposes: lhsT = V (natural layout), rhs = exp(S^T).
  - Softmax denominator comes for free from a ones-column appended to V.
  - Causal masking: upper-triangle j-tiles are skipped entirely (no DMA,
    no matmul); diagonal-stripe tiles get -1e30 folded into the host-side
    bias copy.
  - All matmuls run as float32r (full-rate fp32 on the PE array).
"""

import os
import sys
import numpy as np

for _p in ("/opt/trn_rl_repo", "/root/.axon_site/_ro/trn_rl_repo"):
    if os.path.isdir(_p) and _p not in sys.path:
        sys.path.insert(0, _p)
        break


def _install_ntff_hook():
    """concourse's trace=True path wants antenv.axon_hooks, which the
    image's antenv lacks. Provide it (sys.modules shim) and register the
    ctypes NTFF hook from trn_agent_boot."""
    import types
    try:
        import antenv.axon_hooks  # noqa: F401
        return
    except ImportError:
        pass
    mod = types.ModuleType("antenv.axon_hooks")
    mod._hook = None
    mod.set_axon_ntff_profile_hook = lambda h: setattr(mod, "_hook", h)
    mod.get_axon_ntff_profile_hook = lambda: mod._hook
    try:
        import antenv
        sys.modules["antenv.axon_hooks"] = mod
        antenv.axon_hooks = mod
        from trn_agent_boot.trn_boot import _ntff_profile_via_ctypes
        so = "/opt/axon/libaxon_pjrt.so"
        if os.path.exists(so):
            mod._hook = _ntff_profile_via_ctypes(so)
    except Exception:
        pass


_install_ntff_hook()

# Problem constants (hardcoded per spec).
B, T, D, H = 2, 2048, 1024, 16
HD = D // H            # 64
NCORES = 8
NH = (B * H) // NCORES  # heads per core = 4
DF = NH * HD           # 256  (per-core projection width)
VC = NH * (HD + 1)     # 260  (V with ones-column, 4 heads)
KTILE = 128            # d-dim tile for projections
NKT = D // KTILE       # 8
IC = 512               # query-position chunk (matmul moving dim)
NIC = T // IC          # 4
PJ = 128               # key-position tile (partition dim)
NJT = T // PJ          # 16
NEG = np.float32(-1.0e30)

_STATE = {}
LAST_EXEC_NS = None
LAST_RESULTS = None


def _build_nc():
    import concourse.bass as bass
    import concourse.tile as tile
    from concourse import bacc, mybir
    from contextlib import ExitStack

    F32 = mybir.dt.float32
    F32R = mybir.dt.float32r
    Exp = mybir.ActivationFunctionType.Exp

    nc = bacc.Bacc("TRN2", target_bir_lowering=False, debug=False)

    xqT = nc.dram_tensor("xqT", [D, T], F32R, kind="ExternalInput").ap()
    xkT = nc.dram_tensor("xkT", [D, T], F32R, kind="ExternalInput").ap()
    xvT = nc.dram_tensor("xvT", [D, T], F32R, kind="ExternalInput").ap()
    wqp = nc.dram_tensor("wqp", [NKT + 1, KTILE, DF], F32R, kind="ExternalInput").ap()
    wkp = nc.dram_tensor("wkp", [NKT + 1, KTILE, DF], F32R, kind="ExternalInput").ap()
    wvp = nc.dram_tensor("wvp", [NKT + 1, KTILE, VC], F32R, kind="ExternalInput").ap()
    wot = nc.dram_tensor("wot", [DF, D], F32R, kind="ExternalInput").ap()
    onesd = nc.dram_tensor("onesd", [128, IC], F32R, kind="ExternalInput").ap()
    biasT = nc.dram_tensor("biasT", [NH, T, T], F32, kind="ExternalInput").ap()
    out = nc.dram_tensor("out", [T, D], F32, kind="ExternalOutput").ap()

    r = lambda ap: ap

    with ExitStack() as ctx:
        tc = ctx.enter_context(tile.TileContext(nc))
        consts = ctx.enter_context(tc.tile_pool(name="consts", bufs=1))
        wpool = ctx.enter_context(tc.tile_pool(name="w", bufs=1))
        xpool = ctx.enter_context(tc.tile_pool(name="x", bufs=12))
        qkv = ctx.enter_context(tc.tile_pool(name="qkv", bufs=1))
        bpool = ctx.enter_context(tc.tile_pool(name="bias", bufs=6))
        ppool = ctx.enter_context(tc.tile_pool(name="p", bufs=4))
        outpool = ctx.enter_context(tc.tile_pool(name="outp", bufs=4))
        ppsum = ctx.enter_context(tc.tile_pool(name="ppsum", bufs=2, space="PSUM"))
        spsum = ctx.enter_context(tc.tile_pool(name="spsum", bufs=4, space="PSUM"))
        opsum = ctx.enter_context(tc.tile_pool(name="opsum", bufs=2, space="PSUM"))

        # ones_x: row 0 = 1.0, rows 1..127 = 0. Serves as the "bias row"
        # rhs/lhsT in the K-step 8 of every projection matmul, and as the
        # ones vector for the reciprocal broadcast matmul.
        ones_x = consts.tile([128, IC], F32R, tag="ones")
        nc.sync.dma_start(ones_x, onesd)

        # Weights to SBUF.
        wq_sb = wpool.tile([128, (NKT + 1) * DF], F32R, tag="wq")
        wk_sb = wpool.tile([128, (NKT + 1) * DF], F32R, tag="wk")
        wv_sb = wpool.tile([128, (NKT + 1) * VC], F32R, tag="wv")
        for k in range(NKT + 1):
            nc.sync.dma_start(wq_sb[:, k * DF:(k + 1) * DF], wqp[k])
            nc.sync.dma_start(wk_sb[:, k * DF:(k + 1) * DF], wkp[k])
            nc.sync.dma_start(wv_sb[:, k * VC:(k + 1) * VC], wvp[k])
        wo_sb = [wpool.tile([128, D], F32R, tag=f"wo{m}", name=f"wo{m}") for m in range(2)]
        for m in range(2):
            nc.sync.dma_start(wo_sb[m], wot[m * 128:(m + 1) * 128, :])

        # Persistent activations.
        QT = [qkv.tile([128, T], F32R, tag=f"qt{m}", name=f"qt{m}") for m in range(2)]
        KT = [qkv.tile([128, T], F32R, tag=f"kt{m}", name=f"kt{m}") for m in range(2)]
        Vpp = [qkv.tile([128, VC], F32R, tag=f"vpp{j}", name=f"vpp{j}") for j in range(NJT)]
        OHT = [qkv.tile([128, T], F32R, tag=f"oht{m}", name=f"oht{m}") for m in range(2)]

        # ---- Projections ----
        def load_x(src, cs):
            x_t = []
            for k in range(NKT):
                ks = slice(k * KTILE, (k + 1) * KTILE)
                t_ = xpool.tile([128, IC], F32R, tag="x", name="xt")
                nc.sync.dma_start(t_, src[ks, cs])
                x_t.append(t_)
            return x_t

        for c in range(NIC):
            cs = slice(c * IC, (c + 1) * IC)
            # QT, KT: [f, t] with f on partitions.
            for dst, w_sb, src in ((QT, wq_sb, xqT), (KT, wk_sb, xkT)):
                x_t = load_x(src, cs)
                for m in range(2):
                    ps = ppsum.tile([128, IC], F32, tag="pp")
                    for k in range(NKT + 1):
                        rhs = x_t[k] if k < NKT else ones_x
                        lhsT = w_sb[:, k * DF + m * 128: k * DF + (m + 1) * 128]
                        nc.tensor.matmul(ps, r(lhsT), r(rhs),
                                         start=(k == 0), stop=(k == NKT))
                    nc.scalar.copy(dst[m][:, cs], ps)
            # V'': [t, f] with t on partitions; ones column per head.
            xv_t = load_x(xvT, cs)
            for tt in range(4):
                jt = 4 * c + tt
                ps = ppsum.tile([128, VC], F32, tag="pp")
                for k in range(NKT + 1):
                    lhsT = (xv_t[k][:, tt * 128:(tt + 1) * 128] if k < NKT
                            else ones_x[:, 0:128])
                    rhs = wv_sb[:, k * VC:(k + 1) * VC]
                    nc.tensor.matmul(ps, r(lhsT), r(rhs),
                                     start=(k == 0), stop=(k == NKT))
                nc.scalar.copy(Vpp[jt], ps)

        # ---- Attention (per head, per query chunk) ----
        for h in range(NH):
            mh, rh = h // 2, (h % 2) * 64
            for c in range(NIC):
                cs = slice(c * IC, (c + 1) * IC)
                ps2 = opsum.tile([HD + 1, IC], F32, tag="pv")
                njt = 4 * c + 4  # j-tiles with any unmasked entry
                for jt in range(njt):
                    js = slice(jt * PJ, (jt + 1) * PJ)
                    ps1 = spsum.tile([128, IC], F32, tag="st")
                    nc.tensor.matmul(ps1,
                                     r(KT[mh][rh:rh + 64, js]),
                                     r(QT[mh][rh:rh + 64, cs]),
                                     start=True, stop=True)
                    bt_ = bpool.tile([128, IC], F32, tag="bias")
                    nc.sync.dma_start(bt_, biasT[h, js, cs])
                    nc.vector.tensor_add(ps1, ps1, bt_)
                    pt = ppool.tile([128, IC], F32R, tag="p")
                    nc.scalar.activation(pt, ps1, Exp)
                    nc.tensor.matmul(ps2,
                                     r(Vpp[jt][:, h * (HD + 1):(h + 1) * (HD + 1)]),
                                     r(pt),
                                     start=(jt == 0), stop=(jt == njt - 1))
                # Normalize: row HD of ps2 is the softmax denominator.
                rec = ppool.tile([1, IC], F32R, tag="rec")
                with nc.allow_low_precision(reason="f32r reciprocal feeds PE broadcast"):
                    nc.vector.reciprocal(rec, ps2[HD:HD + 1, :])
                psr = spsum.tile([64, IC], F32, tag="st")
                nc.tensor.matmul(psr, r(ones_x[0:1, 0:64]), r(rec),
                                 start=True, stop=True)
                rep = ppool.tile([64, IC], F32, tag="rep")
                nc.scalar.copy(rep, psr)
                nc.vector.tensor_mul(OHT[mh][rh:rh + 64, cs], ps2[0:HD, :], rep)

        # ---- Output projection (partial over this core's heads) ----
        for tt in range(NJT):
            ts_ = slice(tt * 128, (tt + 1) * 128)
            for e in range(2):
                es = slice(e * IC, (e + 1) * IC)
                ps = ppsum.tile([128, IC], F32, tag="pp")
                for m in range(2):
                    nc.tensor.matmul(ps,
                                     r(OHT[m][:, ts_]),
                                     r(wo_sb[m][:, es]),
                                     start=(m == 0), stop=(m == 1))
                ot = outpool.tile([128, IC], F32, tag="ot")
                nc.scalar.copy(ot, ps)
                nc.sync.dma_start(out[ts_, es], ot)

    nc.compile()
    return nc


def _pack_w(w_aug):
    """[1025, width] -> zero-padded [9, 128, width]."""
    width = w_aug.shape[1]
    out = np.zeros(((NKT + 1) * KTILE, width), np.float32)
    out[:D + 1] = w_aug
    return np.ascontiguousarray(out.reshape(NKT + 1, KTILE, width))


def _prep_core(c, query, key, value, attn_bias, kp_mask,
               wq, bq, wk, bk, wv, bv, wo, xTs):
    b, hg = c // 4, c % 4
    rows = slice(DF * hg, DF * (hg + 1))
    qscale = np.float32(HD ** -0.5)

    wq_aug = np.empty((D + 1, DF), np.float32)
    wq_aug[:D] = wq[rows].T * qscale
    wq_aug[D] = bq[rows] * qscale
    wk_aug = np.empty((D + 1, DF), np.float32)
    wk_aug[:D] = wk[rows].T
    wk_aug[D] = bk[rows]
    wv_aug = np.zeros((D + 1, VC), np.float32)
    wvT = wv[rows].T  # [1024, 256]
    for kh in range(NH):
        wv_aug[:D, kh * (HD + 1):kh * (HD + 1) + HD] = \
            wvT[:, kh * HD:(kh + 1) * HD]
        wv_aug[D, kh * (HD + 1):kh * (HD + 1) + HD] = bv[rows][kh * HD:(kh + 1) * HD]
        wv_aug[D, kh * (HD + 1) + HD] = 1.0

    wot = np.ascontiguousarray(wo[:, rows].T)  # [256, 1024]

    # Host-transposed bias slice: [h, j, i]; fold causal mask (and key
    # padding mask, if any) into the diagonal stripe that the device loads.
    bt = np.ascontiguousarray(
        attn_bias[b, NH * hg:NH * (hg + 1)].transpose(0, 2, 1))
    for jt in range(NJT):
        j0 = jt * PJ
        c0 = IC * (jt // 4)          # first loaded column for this block-row
        width = j0 + PJ - c0
        blk_mask = np.tril(np.ones((PJ, width), bool), k=j0 - c0 - 1)
        blk = bt[:, j0:j0 + PJ, c0:j0 + PJ]
        blk[:, blk_mask] = NEG
    if kp_mask is not None and kp_mask[b].any():
        bt[:, kp_mask[b], :] = NEG

    ones = np.zeros((128, IC), np.float32)
    ones[0, :] = 1.0
    return {
        "xqT": xTs[("q", b)], "xkT": xTs[("k", b)], "xvT": xTs[("v", b)],
        "wqp": _pack_w(wq_aug), "wkp": _pack_w(wk_aug), "wvp": _pack_w(wv_aug),
        "wot": wot, "biasT": bt, "onesd": ones,
    }


def kernel(query, key, value, attn_bias, key_padding_mask,
           wq, bq, wk, bk, wv, bv, wo, bo):
    global LAST_EXEC_NS, LAST_RESULTS
    from concourse.bass_utils import run_bass_kernel_spmd

    query = np.asarray(query, np.float32)
    key = np.asarray(key, np.float32)
    value = np.asarray(value, np.float32)
    attn_bias = np.asarray(attn_bias, np.float32)
    kp = np.asarray(key_padding_mask).astype(bool)
    wq, bq = np.asarray(wq, np.float32), np.asarray(bq, np.float32)
    wk, bk = np.asarray(wk, np.float32), np.asarray(bk, np.float32)
    wv, bv = np.asarray(wv, np.float32), np.asarray(bv, np.float32)
    wo, bo = np.asarray(wo, np.float32), np.asarray(bo, np.float32)

    if "nc" not in _STATE:
        _STATE["nc"] = _build_nc()
    nc = _STATE["nc"]

    xTs = {}
    for tag, arr in (("q", query), ("k", key), ("v", value)):
        for b in range(B):
            xTs[(tag, b)] = np.ascontiguousarray(arr[b].T)

    from concurrent.futures import ThreadPoolExecutor
    with ThreadPoolExecutor(NCORES) as ex:
        in_maps = list(ex.map(
            lambda c: _prep_core(c, query, key, value, attn_bias, kp,
                                 wq, bq, wk, bk, wv, bv, wo, xTs),
            range(NCORES)))

    trace = os.environ.get("BASS_KERNEL_TRACE", "0") == "1"
    res = run_bass_kernel_spmd(nc, in_maps, core_ids=list(range(NCORES)),
                               trace=trace)
    LAST_EXEC_NS = res.exec_time_ns
    LAST_RESULTS = res

    out = np.empty((B, T, D), np.float32)
    for b in range(B):
        acc = res.results[4 * b]["out"].astype(np.float32)
        for g in range(1, 4):
            acc = acc + res.results[4 * b + g]["out"]
        out[b] = acc + bo
    return out
